# revision 55
# baseline (speedup 1.0000x reference)
"""Causal self-attention (B=1, T=2048, C=1024, H=16, RoPE) on 8 TRN2 NeuronCores.

Sharding: 2 heads per core (tensor parallel on w_qkv columns / w_proj rows).
Each core computes a full-shape partial output; the host sums the 8 partials
(the tensor-parallel all-reduce, done at gather time).

v2 vs baseline (110us): bf16 for QKV/S/PV matmuls, 1024-query chunks with
wide exp instructions (halves ACT-engine overhead), q/k transposes moved
from PE+ACT to the DMA XBAR, causal masks as a prebuilt triangle multiply
on DVE, proj PSUM copies split DVE/Pool. PE stream is emission-ordered so
QKV/proj matmuls fill the gaps where attention waits on exp.

Per-core pipeline:
  - host ships x as (128, 8k, T) bf16 tiles (pre-transposed), w_qkv columns
    permuted per head to (evens|odds) so rope halves are contiguous
  - qkv = x @ w_local in bf16 (fp32 PSUM accumulate)
  - RoPE on q,k in natural layout (DVE muls + gpsimd combines), bf16 out
  - q,k transposed to (dim, T) via DMA XBAR transpose (bf16)
  - scores transposed: S^T[j,i] = k_j . q_i over 1024-query chunks; exp on
    ACT (scale=1/8); denominators free via a ones-column in V (65-row PV)
  - causal masking: multiply diagonal 128x128 blocks by a triangle tile
  - normalize via DVE reciprocal + gpsimd partition_broadcast
  - partial_out = attn_out @ w_proj_local in fp32r
"""

import numpy as np

B, T, C, H = 1, 2048, 1024, 16
D = C // H  # 64
ROPE_THETA = 10000.0
N_CORES = 8
MT = T // 128  # 16 m-tiles
NCH = 2        # query chunks of 1024
MPC = 8        # m-tiles per chunk

_CACHE = {}


def build_module(repeat=1, debug_taps=False):
    import concourse.bass as bass
    import concourse.mybir as mybir
    import concourse.tile as tile
    from concourse import bacc

    f32 = mybir.dt.float32
    f32r = mybir.dt.float32r
    bf16 = mybir.dt.bfloat16
    EXP = mybir.ActivationFunctionType.Exp
    GE = mybir.AluOpType.is_ge

    nc = bacc.Bacc("TRN2", target_bir_lowering=False, debug=False,
                   num_devices=N_CORES)

    xT_in = nc.declare_dram_parameter("xT_in", [128, 8, T], bf16, isOutput=False)
    w_l = nc.declare_dram_parameter("w_l", [C, 384], bf16, isOutput=False)
    wp_l = nc.declare_dram_parameter("wp_l", [128, C], f32r, isOutput=False)
    # cos/sin pre-tiled host-side to (128, 16, 32) so the DMA lines are 2KB
    cos_t = nc.declare_dram_parameter("cos_t", [128, MT, 32], f32,
                                      isOutput=False)
    sin_t = nc.declare_dram_parameter("sin_t", [128, MT, 32], f32,
                                      isOutput=False)
    out_p = nc.declare_dram_parameter("out_p", [T, C], bf16, isOutput=True)
    if debug_taps:
        dbg_rqk = nc.declare_dram_parameter("dbg_rqk", [128, MPC, 2, 128],
                                            bf16, isOutput=True)
        dbg_qkT = nc.declare_dram_parameter("dbg_qkT", [128, 2, MPC, 128],
                                            bf16, isOutput=True)
        dbg_v = nc.declare_dram_parameter("dbg_v", [128, 130], bf16,
                                          isOutput=True)
        dbg_e = nc.declare_dram_parameter("dbg_e", [128, 1024], bf16,
                                          isOutput=True)
        dbg_a = nc.declare_dram_parameter("dbg_a", [128, 1024], f32r,
                                          isOutput=True)

    import contextlib

    with tile.TileContext(nc) as tc:
        with tc.tile_pool(name="singles", bufs=1) as singles, \
             (tc.For_i(0, repeat, 1,
                       hint_engines=(mybir.EngineType.PE,
                                     mybir.EngineType.Activation,
                                     mybir.EngineType.DVE,
                                     mybir.EngineType.Pool,
                                     mybir.EngineType.SP))
              if repeat > 1 else contextlib.nullcontext()):
            # ---- static SBUF tensors; DMA order tuned for startup latency:
            # first QKV matmul needs w k0-3 + x cols [0:128] only.
            w_sb = singles.tile([128, 8, 384], bf16)
            w_r = w_l.ap().rearrange("(kt p) n -> p kt n", p=128)
            xT_sb = singles.tile([128, 8, T], bf16)

            def dma_x(c0, c1):
                nc.sync.dma_start(out=xT_sb[:, :, c0:c1],
                                  in_=xT_in[:, :, c0:c1])

            # All bulk input DMAs dispatch first on SP; the dependency-gated
            # q/k XBAR transposes are emitted after (same SP lane) so they
            # never head-of-line-block a bulk transfer.
            nc.sync.dma_start(out=w_sb[:, 0:2, :], in_=w_r[:, 0:2, :])
            dma_x(0, 256)
            nc.sync.dma_start(out=w_sb[:, 2:8, :], in_=w_r[:, 2:8, :])
            dma_x(256, 512)
            cos_sb = singles.tile([128, MT, 32], f32)
            nc.sync.dma_start(out=cos_sb, in_=cos_t.ap())
            sin_sb = singles.tile([128, MT, 32], f32)
            nc.sync.dma_start(out=sin_sb, in_=sin_t.ap())
            dma_x(512, 768)
            dma_x(768, 1024)
            dma_x(1024, 1536)
            dma_x(1536, 2048)
            wp_sb = singles.tile([128, 1024], f32r)
            nc.sync.dma_start(out=wp_sb, in_=wp_l[:, :])

            # triangle mask: tri[jj, c] = 1 if c >= jj else 0 (bf16)
            tri_f = singles.tile([128, 128], f32)
            nc.gpsimd.memset(tri_f, 1.0)
            nc.gpsimd.affine_select(
                out=tri_f, in_=tri_f, compare_op=GE, fill=0.0, base=0,
                pattern=[[1, 128]], channel_multiplier=-1)
            tri_sb = singles.tile([128, 128], bf16)
            nc.vector.tensor_copy(tri_sb, tri_f)

            # identity for PE transposes (f32r, as in the proven baseline)
            ident = singles.tile([128, 128], f32r)
            nc.gpsimd.memset(ident.bitcast(f32), 0.0)
            nc.gpsimd.affine_select(
                out=ident, in_=ident, compare_op=mybir.AluOpType.not_equal,
                fill=1.0, base=0, pattern=[[-1, 128]], channel_multiplier=1)

            # per-chunk transposed q/k (one tile: [d, qk, m, t]); rope
            # staging in f32r for the PE transpose; per-j-tile v
            qkT = [singles.tile([128, 2, MPC, 128], bf16, name=f"qkT{c}",
                                tag=f"qkT{c}") for c in range(NCH)]
            rqk = [singles.tile([128, MPC, 2, 128], f32r, name=f"rqk{c}",
                                tag=f"rqk{c}") for c in range(NCH)]
            v_t = [singles.tile([128, 130], bf16, name=f"v{j}", tag=f"v{j}")
                   for j in range(MT)]
            for j in range(MT):
                nc.vector.memset(v_t[j][:, 64:65], 1.0)
                nc.vector.memset(v_t[j][:, 129:130], 1.0)
            aT = [singles.tile([128, 1024], f32r, name=f"aT{c}", tag=f"aT{c}")
                  for c in range(NCH)]

            with \
                 tc.tile_pool(name="mm_ps", bufs=3, space="PSUM") as mmpool, \
                 tc.tile_pool(name="s2_ps", bufs=3, space="PSUM") as s2pool, \
                 tc.tile_pool(name="o_ps", bufs=2, space="PSUM") as opool, \
                 tc.tile_pool(name="rope_tmp", bufs=4) as tmppool, \
                 tc.tile_pool(name="e_sb", bufs=4) as epool, \
                 tc.tile_pool(name="r_sb", bufs=4) as rpool, \
                 tc.tile_pool(name="rb_sb", bufs=4) as rbpool, \
                 tc.tile_pool(name="o_out", bufs=3) as ospool:

                def emit_qkv(m):
                    ch, mo = m // MPC, m % MPC
                    qkv_ps = mmpool.tile([128, 384], f32, name=f"qkv{m}",
                                         tag="mm")
                    for k in range(8):
                        nc.tensor.matmul(
                            qkv_ps,
                            xT_sb[:, k, 128 * m:128 * m + 128],
                            w_sb[:, k, :],
                            start=(k == 0), stop=(k == 7))

                    # v copy emitted first so it clears the moment the PSUM
                    # tile stops: cols [0:64] and [65:129] in one strided op.
                    # (GPSIMD cannot read PSUM, so this is on DVE.)
                    v_dst = bass.AP(tensor=v_t[m].tensor,
                                    offset=v_t[m].offset,
                                    ap=[v_t[m].ap[0], [65, 2], [1, 64]])
                    nc.vector.tensor_copy(
                        v_dst,
                        qkv_ps[:, 256:384].rearrange("p (b d) -> p b d", b=2))

                    # RoPE: multiply q,k (cols 0:256) by cos and sin tables
                    cos_b = bass.AP(tensor=cos_sb.tensor,
                                    offset=cos_sb[:, m, :].offset,
                                    ap=[cos_sb.ap[0], [0, 8], [1, 32]])
                    sin_b = bass.AP(tensor=sin_sb.tensor,
                                    offset=sin_sb[:, m, :].offset,
                                    ap=[sin_sb.ap[0], [0, 8], [1, 32]])
                    src8 = qkv_ps[:, 0:256].rearrange("p (b d) -> p b d", b=8)
                    tcos = tmppool.tile([128, 8, 32], f32, name=f"tc_{m}",
                                        tag="tc")
                    tsin = tmppool.tile([128, 8, 32], f32, name=f"ts_{m}",
                                        tag="ts")
                    nc.vector.tensor_mul(tcos, src8, cos_b)
                    nc.vector.tensor_mul(tsin, src8, sin_b)
                    # combine rotated halves into rqk[ch][:, mo, qk, (h eo d)]
                    rv = rqk[ch].rearrange("p m q (h e d) -> p m q h e d",
                                           h=2, e=2)
                    tc4 = tcos.rearrange("p (q h e) d -> p q h e d", q=2, h=2)
                    ts4 = tsin.rearrange("p (q h e) d -> p q h e d", q=2, h=2)
                    nc.gpsimd.tensor_sub(rv[:, mo, :, :, 0, :],
                                         tc4[:, :, :, 0, :], ts4[:, :, :, 1, :])
                    nc.gpsimd.tensor_add(rv[:, mo, :, :, 1, :],
                                         ts4[:, :, :, 0, :], tc4[:, :, :, 1, :])

                    # q,k transposes on PE (f32r, baseline-proven); the
                    # PSUM->SBUF copies cast to bf16 on ACT and DVE
                    tp = mmpool.tile([128, 256], f32r, name=f"tp{m}",
                                     tag="mm")
                    nc.tensor.transpose(tp[:, 0:128], rqk[ch][:, mo, 0, :],
                                        ident)
                    nc.tensor.transpose(tp[:, 128:256], rqk[ch][:, mo, 1, :],
                                        ident)
                    nc.scalar.copy(qkT[ch][:, 0, mo, :], tp[:, 0:128])
                    nc.vector.tensor_copy(qkT[ch][:, 1, mo, :],
                                          tp[:, 128:256])

                def emit_S(ic, h, jt):
                    """S^T then per-half exp for j-tile jt, query chunk ic.

                    Each 512-query half gets its own 1-bank PSUM tile and its
                    own exp: exp_a runs on ACT while PE is still doing S_b,
                    and each bank frees as soon as its half is exp'd.
                    """
                    mm = jt - MPC * ic
                    e2 = epool.tile([128, 1024], bf16, name=f"e{ic}_{h}_{jt}",
                                    tag="e")
                    lhsT = qkT[jt // MPC][64 * h:64 * h + 64, 1, jt % MPC, :]
                    q2 = qkT[ic][:, 0, :, :].rearrange("p m d -> p (m d)")
                    q0 = max(0, 128 * mm)
                    if q0 < 512:
                        sa = s2pool.tile([128, 512], f32,
                                         name=f"sa{ic}_{h}_{jt}", tag="s2")
                        nc.tensor.matmul(sa[:, q0:512], lhsT,
                                         q2[64 * h:64 * h + 64, q0:512],
                                         start=True, stop=True)
                        nc.scalar.activation(e2[:, q0:512], sa[:, q0:512],
                                             EXP, scale=0.125)
                        if mm >= 0:
                            nc.vector.tensor_mul(e2[:, q0:q0 + 128],
                                                 e2[:, q0:q0 + 128], tri_sb)
                        sb = s2pool.tile([128, 512], f32,
                                         name=f"sb{ic}_{h}_{jt}", tag="s2")
                        nc.tensor.matmul(sb, lhsT,
                                         q2[64 * h:64 * h + 64, 512:1024],
                                         start=True, stop=True)
                        nc.scalar.activation(e2[:, 512:1024], sb,
                                             EXP, scale=0.125)
                    else:
                        sb = s2pool.tile([128, 512], f32,
                                         name=f"sb{ic}_{h}_{jt}", tag="s2")
                        nc.tensor.matmul(sb[:, q0 - 512:512], lhsT,
                                         q2[64 * h:64 * h + 64, q0:1024],
                                         start=True, stop=True)
                        nc.scalar.activation(e2[:, q0:1024],
                                             sb[:, q0 - 512:512],
                                             EXP, scale=0.125)
                        nc.vector.tensor_mul(e2[:, q0:q0 + 128],
                                             e2[:, q0:q0 + 128], tri_sb)
                    if debug_taps and (ic, h, jt) == (0, 0, 0):
                        nc.sync.dma_start(out=dbg_e.ap(), in_=e2)
                    return e2

                def emit_PV(ic, h, jt, e2, oA, oB):
                    """PV accumulate; the causally-masked diagonal 128-block
                    is a separate matmul so the bulk doesn't wait on the DVE
                    mask multiply."""
                    mm = jt - MPC * ic
                    jA_stop = MPC * ic + 3   # last j-tile feeding half A
                    jB_stop = MPC * ic + 7
                    lhsT = v_t[jt][:, 65 * h:65 * h + 65]
                    st, spA, spB = (jt == 0), (jt == jA_stop), (jt == jB_stop)
                    q0 = max(0, 128 * mm)
                    if mm < 0:
                        nc.tensor.matmul(oA, lhsT, e2[:, 0:512],
                                         start=st, stop=spA,
                                         skip_group_check=True)
                        nc.tensor.matmul(oB, lhsT, e2[:, 512:1024],
                                         start=st, stop=spB,
                                         skip_group_check=True)
                    elif q0 < 512:
                        if st:
                            # single full write: two start=True matmuls into
                            # one PSUM bank reset each other's accumulation
                            nc.tensor.matmul(oB, lhsT, e2[:, 512:1024],
                                             start=True, stop=spB,
                                             skip_group_check=True)
                            nc.tensor.matmul(oA[:, q0:512], lhsT,
                                             e2[:, q0:512],
                                             start=True, stop=spA,
                                             skip_group_check=True)
                        else:
                            if q0 + 128 < 512:  # bulk of A (exp-gated only)
                                nc.tensor.matmul(oA[:, q0 + 128:512], lhsT,
                                                 e2[:, q0 + 128:512],
                                                 start=False, stop=spA,
                                                 skip_group_check=True)
                            nc.tensor.matmul(oB, lhsT, e2[:, 512:1024],
                                             start=False, stop=spB,
                                             skip_group_check=True)
                            nc.tensor.matmul(oA[:, q0:q0 + 128], lhsT,
                                             e2[:, q0:q0 + 128],
                                             start=False, stop=spA,
                                             skip_group_check=True)
                    else:
                        if q0 + 128 < 1024:
                            nc.tensor.matmul(oB[:, q0 - 512 + 128:512], lhsT,
                                             e2[:, q0 + 128:1024],
                                             start=st, stop=spB,
                                             skip_group_check=True)
                        nc.tensor.matmul(oB[:, q0 - 512:q0 - 512 + 128], lhsT,
                                         e2[:, q0:q0 + 128],
                                         start=st, stop=spB,
                                         skip_group_check=True)

                def emit_norm_half(ic, h, o_h, half):
                    r_t = rpool.tile([1, 512], f32, name=f"r{ic}_{h}_{half}",
                                     tag="r")
                    nc.vector.reciprocal(r_t, o_h[64:65, :])
                    rb_t = rbpool.tile([64, 512], f32, name=f"rb{ic}_{h}_{half}",
                                       tag="rb")
                    nc.gpsimd.partition_broadcast(rb_t, r_t[0:1, :],
                                                  channels=64)
                    nc.vector.tensor_mul(
                        aT[ic][64 * h:64 * h + 64, 512 * half:512 * half + 512],
                        o_h[0:64, :], rb_t)

                def emit_proj(m, tail=False):
                    ch, mo = m // MPC, m % MPC
                    o_sb = ospool.tile([128, 1024], bf16, name=f"os{m}",
                                       tag="os")
                    a_l = aT[ch][:, 128 * mo:128 * mo + 128]
                    for n2 in range(2):
                        p_ps = mmpool.tile([128, 512], f32,
                                           name=f"p{m}_{n2}", tag="mm")
                        nc.tensor.matmul(p_ps, a_l,
                                         wp_sb[:, 512 * n2:512 * n2 + 512],
                                         start=True, stop=True)
                        dst = o_sb[:, 512 * n2:512 * n2 + 512]
                        if n2 == 0:
                            nc.vector.tensor_copy(dst, p_ps)
                        elif tail or m % 2 == 1:
                            nc.scalar.copy(dst, p_ps)
                        else:
                            nc.vector.tensor_copy(dst, p_ps)
                    nc.sync.dma_start(
                        out=out_p.ap()[128 * m:128 * m + 128, :], in_=o_sb)

                # ---------------- emission schedule ----------------
                for m in range(0, 10):
                    emit_qkv(m)

                # chunk 0 attention, heads sequential; fill with qkv m10..15
                fillers = list(range(10, 16))

                def pop_filler():
                    if fillers:
                        emit_qkv(fillers.pop(0))

                for h in range(2):
                    oA = opool.tile([65, 512], f32, name=f"oA0_{h}", tag="o")
                    oB = opool.tile([65, 512], f32, name=f"oB0_{h}", tag="o")
                    pend = []
                    for jt in range(MPC):
                        e2 = emit_S(0, h, jt)
                        pend.append((jt, e2))
                        if len(pend) > 2:
                            j0, e0 = pend.pop(0)
                            emit_PV(0, h, j0, e0, oA, oB)
                            if j0 == 3:  # half A complete: free oA early
                                emit_norm_half(0, h, oA, 0)
                        if jt % 2 == 0:
                            pop_filler()
                    for j0, e0 in pend:
                        emit_PV(0, h, j0, e0, oA, oB)
                    emit_norm_half(0, h, oB, 1)
                    pop_filler()
                while fillers:
                    pop_filler()

                # chunk 1 attention; fill with proj m0..7 (chunk-0 output)
                # spread over both heads; proj m8..11 (half A of chunk 1)
                # interleave once both heads' half-A norms land; m12..15 tail.
                proj_fill = [[0, 1, 2, 3], [4, 5, 6, 7]]
                normA_done = [False, False]
                late = []

                def pop_proj(h):
                    if proj_fill[h]:
                        emit_proj(proj_fill[h].pop(0))
                    elif late:
                        emit_proj(late.pop(0))

                for h in range(2):
                    oA = opool.tile([65, 512], f32, name=f"oA1_{h}", tag="o")
                    oB = opool.tile([65, 512], f32, name=f"oB1_{h}", tag="o")
                    pend = []
                    for jt in range(2 * MPC):
                        e2 = emit_S(1, h, jt)
                        pend.append((jt, e2))
                        if len(pend) > 2:
                            j0, e0 = pend.pop(0)
                            emit_PV(1, h, j0, e0, oA, oB)
                            if j0 == MPC + 3:  # half A complete
                                emit_norm_half(1, h, oA, 0)
                                normA_done[h] = True
                                if all(normA_done):
                                    late.extend((8, 9, 10, 11))
                        if h == 0 and jt % 3 == 1:
                            pop_proj(h)
                        elif h == 1 and (jt % 2 == 1 or jt >= 12):
                            pop_proj(h)
                    for j0, e0 in pend:
                        emit_PV(1, h, j0, e0, oA, oB)
                    emit_norm_half(1, h, oB, 1)
                    while proj_fill[h]:
                        pop_proj(h)
                while late:
                    emit_proj(late.pop(0))
                for m in (12, 13, 14, 15):
                    emit_proj(m, tail=True)

                if debug_taps:
                    nc.sync.dma_start(out=dbg_qkT.ap(), in_=qkT[0][:, :, :, :])
                    nc.sync.dma_start(out=dbg_v.ap(), in_=v_t[0][:, :])
                    nc.sync.dma_start(out=dbg_a.ap(), in_=aT[0][:, :])

    nc.compile()
    return nc


def host_inputs(x, w_qkv, w_proj):
    """Build per-core input maps from the full inputs."""
    import ml_dtypes
    bf16 = ml_dtypes.bfloat16

    x2 = np.asarray(x, dtype=np.float32).reshape(T, C)
    # (128, 8, T): xT_in[p, k, t] = x[t, 128k + p]
    xT = np.ascontiguousarray(
        x2.T.reshape(8, 128, T).transpose(1, 0, 2)).astype(bf16)
    wq = np.asarray(w_qkv, dtype=np.float32)
    wp = np.asarray(w_proj, dtype=np.float32)

    inv_freq = 1.0 / (ROPE_THETA ** (np.arange(0, D, 2, dtype=np.float32) / D))
    ang = np.arange(T, dtype=np.float32)[:, None] * inv_freq[None, :]
    # pre-tiled (128, MT, 32): cos_t[p, mt, d] = cos(ang[mt*128 + p, d])
    cos_t = np.ascontiguousarray(
        np.cos(ang).astype(np.float32).reshape(MT, 128, 32).transpose(1, 0, 2))
    sin_t = np.ascontiguousarray(
        np.sin(ang).astype(np.float32).reshape(MT, 128, 32).transpose(1, 0, 2))

    perm = np.concatenate([np.arange(0, D, 2), np.arange(1, D, 2)])  # evens|odds

    in_maps = []
    for c in range(N_CORES):
        h0, h1 = 2 * c, 2 * c + 1
        cols = []
        for h in (h0, h1):      # q blocks, permuted
            cols.append(wq[:, h * D:(h + 1) * D][:, perm])
        for h in (h0, h1):      # k blocks, permuted
            cols.append(wq[:, C + h * D:C + (h + 1) * D][:, perm])
        for h in (h0, h1):      # v blocks, natural
            cols.append(wq[:, 2 * C + h * D:2 * C + (h + 1) * D])
        w_l = np.ascontiguousarray(
            np.concatenate(cols, axis=1)).astype(bf16)   # (1024, 384)
        wp_l = np.ascontiguousarray(wp[128 * c:128 * c + 128, :])  # (128, 1024)
        in_maps.append({
            "xT_in": xT, "w_l": w_l, "wp_l": wp_l,
            "cos_t": cos_t, "sin_t": sin_t,
        })
    return in_maps


def kernel(x, w_qkv, w_proj):
    from concourse.bass_utils import run_bass_kernel_spmd

    if "nc" not in _CACHE:
        _CACHE["nc"] = build_module()
    nc = _CACHE["nc"]

    in_maps = host_inputs(x, w_qkv, w_proj)
    res = run_bass_kernel_spmd(nc, in_maps, list(range(N_CORES)))
    out = np.zeros((T, C), dtype=np.float32)
    for c in range(N_CORES):
        out += res.results[c]["out_p"].astype(np.float32)
    return out.reshape(B, T, C)
